# revision 5
# baseline (speedup 1.0000x reference)
"""Trainium2 Bass kernel for nn_CRAP_16544214024675 (sparse_attention).

Reference computation (per batch b, channel c):
  q = Wq@feat + bq                        (1x1 conv over channels)
  k = unfold3x3_s2(src)                   (strided window gather, pad 1)
  v = unfold3x3_s2(Wv@src + bv)
  A = softmax_t( sum_px q*k_t / 64 )      (9 window positions)
  out = fold3x3_s1( A_t * v_t ) * feat

Sharding: 8 cores = 4 batches x 2 output-channel halves. The per-core
program is identical: inputs are packed so channel-slot 0 is always the
core's OWN half (host reorders the contraction; channel sums commute).

Layout trick: unfold/fold never materialize. With parity planes
P[p,q][c,h,w] = x[c, 2h+p, 2w+q]:
  k_(i,j)[h,w] = src[2h+i-1, 2w+j-1] -> srcplane[(i+1)%2][(j+1)%2]
    shifted by (i==0 ? -1 : 0, j==0 ? -1 : 0)
  fold sample for t=(i,j) at (y,x) = vsrc[2y+1-i, 2x+1-j]
    -> vplane[(i+1)%2][(j+1)%2][y + (i==2 ? -1 : 0), x + (j==2 ? -1 : 0)]
Out-of-range samples are the zero-pad terms; every op restricts to its
valid window so they drop out exactly (incl. the fold's dropped
(y=63,i=0)/(x=63,j=0) terms; PSUM has_written handles partial windows
because the full-window t=(1,1) runs first with start=True).

Engines:
  PE : warm-up burst, q-conv, v-conv, fold (diag(exp_t) stationary,
       PSUM-accumulated over the 9 t's)
  DVE: logits = scalar_tensor_tensor (q*k_t with fused accum_out sum),
       diag builds, 1/Z, final (fold*1/Z)*feat fused multiply
  ACT: PSUM->SBUF copies with fused bias (Identity), exp(logit/64),
       q column-shift copy
DMA is issued from sync+gpsimd in consumer order (feat, then src planes
in first-use order, featf last).
"""
import sys
from contextlib import ExitStack

import numpy as np

for _p in ("/opt/trn_rl_repo", "/root/.axon_site/_ro/trn_rl_repo"):
    if _p not in sys.path:
        sys.path.append(_p)

import ml_dtypes

import concourse.tile as tile
from concourse import bacc, mybir
from concourse import bass_utils
from concourse.bass_interp import get_hw_module

F32 = mybir.dt.float32
BF16 = mybir.dt.bfloat16
AF = mybir.ActivationFunctionType
ALU = mybir.AluOpType

B, C, H, W = 4, 256, 64, 64
N_CORES = 8

# (1,1) first (full-window fold matmul initializes PSUM with start=True);
# the three j==0 entries (which need the shifted q copy) go last so the
# ACT-produced shift is ready off the critical path.
T_ORDER = [(1, 1), (0, 1), (0, 2), (1, 2), (2, 1), (2, 2), (0, 0), (1, 0), (2, 0)]
# srcplane/vplane first-use order for t in T_ORDER: (p,q)=((i+1)%2,(j+1)%2)
PLANE_ORDER = [(0, 0), (1, 0), (1, 1), (0, 1)]


def build_program():
    nc = bacc.Bacc("TRN2", target_bir_lowering=False, debug=False)

    featb_d = nc.dram_tensor("featb", (2, 128, H, W), BF16, kind="ExternalInput")
    featf_d = nc.dram_tensor("featf", (128, H, W), F32, kind="ExternalInput")
    spl_d = nc.dram_tensor("splanes", (2, 2, 2, 128, H, W), BF16, kind="ExternalInput")
    wq_d = nc.dram_tensor("wq", (2, 128, 128), BF16, kind="ExternalInput")
    wv_d = nc.dram_tensor("wv", (2, 128, 128), BF16, kind="ExternalInput")
    bq_d = nc.dram_tensor("bq", (128, 1), F32, kind="ExternalInput")
    bv_d = nc.dram_tensor("bv", (128, 1), F32, kind="ExternalInput")
    id_d = nc.dram_tensor("identb", (128, 128), BF16, kind="ExternalInput")
    out_d = nc.dram_tensor("out", (128, H, W), F32, kind="ExternalOutput")

    with tile.TileContext(nc) as tc, ExitStack() as ctx:
        pool = ctx.enter_context(tc.tile_pool(name="main", bufs=1))
        scpool = ctx.enter_context(tc.tile_pool(name="scratch", bufs=1))
        dgpool = ctx.enter_context(tc.tile_pool(name="diags", bufs=9))

        # --- weights / constants (gpsimd queue; needed by PE warm-up) ---
        wq_t = pool.tile([128, 2, 128], BF16, tag="wq")
        wv_t = pool.tile([128, 2, 128], BF16, tag="wv")
        for k in range(2):
            nc.gpsimd.dma_start(wq_t[:, k, :], wq_d.ap()[k])
            nc.gpsimd.dma_start(wv_t[:, k, :], wv_d.ap()[k])
        bq_t = pool.tile([128, 1], F32, tag="bq")
        nc.gpsimd.dma_start(bq_t[:], bq_d.ap())
        bv_t = pool.tile([128, 1], F32, tag="bv")
        nc.gpsimd.dma_start(bv_t[:], bv_d.ap())
        id_t = pool.tile([128, 128], BF16, tag="identb")
        nc.gpsimd.dma_start(id_t[:], id_d.ap())

        # --- feat (bf16, slot 0 = own half) ---
        featb_t = []
        for k in range(2):
            t_ = pool.tile([128, H, W], BF16, tag=f"featb{k}")
            (nc.sync if k == 0 else nc.gpsimd).dma_start(t_[:], featb_d.ap()[k])
            featb_t.append(t_)

        # --- src parity planes, first-use order, alternating issue queues ---
        splane = [[[None] * 2 for _ in range(2)] for _ in range(2)]
        eng_rr = [nc.sync, nc.gpsimd]
        n_dma = 0
        for (p, q) in PLANE_ORDER:
            for ct in range(2):
                t_ = pool.tile(
                    [128, H, W], BF16, tag=f"spl{ct}{p}{q}", name=f"spl{ct}{p}{q}"
                )
                eng_rr[n_dma % 2].dma_start(t_[:], spl_d.ap()[ct, p, q])
                splane[ct][p][q] = t_
                n_dma += 1

        with tc.tile_pool(name="psq", bufs=2, space="PSUM") as psq:
            # --- PE warm-up: ~5us of dummy matmuls so the HAM clock-gate
            # opens before the real convs (weights land in ~1us) ---
            wps = psq.tile([128, 32, W], F32, tag="ps", name="warmps")
            for w_i in range(24):
                nc.tensor.matmul(
                    wps[:, 0:4, :],
                    wq_t[:, 0, :],
                    wq_t[:].rearrange("p a b -> p (a b)"),
                    start=True,
                    stop=True,
                    skip_group_check=True,
                )

            # --- q-conv: q = Wq@feat + bq -> bf16 (bias fused on ACT copy) ---
            q_t = pool.tile([128, H, W], BF16, tag="q")
            qs_t = pool.tile([128, H, W], BF16, tag="qs")
            for half in range(2):
                ps = psq.tile([128, 32, W], F32, tag="ps")
                r0 = 32 * half
                for s in range(4):
                    for k in range(2):
                        nc.tensor.matmul(
                            ps[:, 8 * s : 8 * s + 8, :],
                            wq_t[:, k, :],
                            featb_t[k][:, r0 + 8 * s : r0 + 8 * s + 8, :],
                            start=(k == 0),
                            stop=(k == 1),
                        )
                nc.scalar.activation(
                    q_t[:, r0 : r0 + 32, :], ps[:], AF.Identity, bias=bq_t[:]
                )
            # shifted q copy (ACT; used only by the three late j==0 t's)
            nc.scalar.activation(qs_t[:, :, 0:63], q_t[:, :, 1:64], AF.Copy)

            # --- v-conv: vplane[p][q] = Wv@srcplane[:, p, q] + bv ---
            vplane = [[None] * 2 for _ in range(2)]
            for (p, q) in PLANE_ORDER:
                vplane[p][q] = pool.tile(
                    [128, H, W], BF16, tag=f"vpl{p}{q}", name=f"vpl{p}{q}"
                )
                for half in range(2):
                    ps = psq.tile([128, 32, W], F32, tag="ps")
                    r0 = 32 * half
                    for s in range(4):
                        for k in range(2):
                            nc.tensor.matmul(
                                ps[:, 8 * s : 8 * s + 8, :],
                                wv_t[:, k, :],
                                splane[k][p][q][:, r0 + 8 * s : r0 + 8 * s + 8, :],
                                start=(k == 0),
                                stop=(k == 1),
                            )
                    nc.scalar.activation(
                        vplane[p][q][:, r0 : r0 + 32, :],
                        ps[:],
                        AF.Identity,
                        bias=bv_t[:],
                    )

        # --- per-t pipeline: fused logit (DVE) -> exp (ACT) -> diag (DVE)
        #     -> fold matmuls (PE, PSUM-accumulated) ---
        psf = ctx.enter_context(tc.tile_pool(name="psf", bufs=1, space="PSUM"))
        lg_t = pool.tile([128, 9], F32, tag="lg")
        exp_t = pool.tile([128, 9], F32, tag="exp")
        fold_ps = psf.tile([128, H, W], F32, tag="fold")
        sc = scpool.tile([128, H, W], BF16, tag="prod")

        for idx, (i, j) in enumerate(T_ORDER):
            if i == 0:
                qr0, rows = 1, 63
            else:
                qr0, rows = 0, 64
            if j == 0:
                qq, cols = qs_t, 63
            else:
                qq, cols = q_t, 64
            pl = splane[0][(i + 1) % 2][(j + 1) % 2]
            nc.vector.scalar_tensor_tensor(
                out=sc[:, 0:rows, 0:cols],
                in0=qq[:, qr0 : qr0 + rows, 0:cols],
                scalar=1.0,
                in1=pl[:, 0:rows, 0:cols],
                op0=ALU.mult,
                op1=ALU.mult,
                accum_out=lg_t[:, idx : idx + 1],
            )
            nc.scalar.activation(
                exp_t[:, idx : idx + 1],
                lg_t[:, idx : idx + 1],
                AF.Exp,
                scale=1.0 / 64.0,
            )
            dg = dgpool.tile([128, 128], BF16, tag="diag", name=f"diag{idx}")
            nc.vector.tensor_scalar_mul(dg[:], id_t[:], exp_t[:, idx : idx + 1])

            # fold windows: psum[y,x] += exp_t * vplane[pi][pj][y+dy, x+dx]
            if i == 0:
                yo0, yo1, dy = 0, 63, 0
            elif i == 1:
                yo0, yo1, dy = 0, 64, 0
            else:
                yo0, yo1, dy = 1, 64, -1
            if j == 0:
                xo0, xo1, dx = 0, 63, 0
            elif j == 1:
                xo0, xo1, dx = 0, 64, 0
            else:
                xo0, xo1, dx = 1, 64, -1
            vp = vplane[(i + 1) % 2][(j + 1) % 2]
            yb = yo0
            while yb < yo1:
                ye = min((yb // 8 + 1) * 8, yo1)
                nc.tensor.matmul(
                    fold_ps[:, yb:ye, xo0:xo1],
                    dg[:],
                    vp[:, yb + dy : ye + dy, xo0 + dx : xo1 + dx],
                    start=(idx == 0),
                    stop=(idx == 8),
                    skip_group_check=True,
                )
                yb = ye

        # --- 1/Z off the critical path ---
        z_t = pool.tile([128, 1], F32, tag="z")
        rz_t = pool.tile([128, 1], F32, tag="rz")
        nc.vector.tensor_reduce(z_t[:], exp_t[:], axis=mybir.AxisListType.X, op=ALU.add)
        nc.vector.reciprocal(rz_t[:], z_t[:])

        # --- final: out = (fold * 1/Z) * feat, two halves pipelined ---
        fs_t = pool.tile([128, H, W], F32, tag="fs")
        nc.gpsimd.dma_start(fs_t[:], featf_d.ap())
        out_t = pool.tile([128, H, W], F32, tag="out")
        for half in range(2):
            r0 = 32 * half
            nc.vector.scalar_tensor_tensor(
                out=out_t[:, r0 : r0 + 32, :],
                in0=fold_ps[:, r0 : r0 + 32, :],
                scalar=rz_t[:],
                in1=fs_t[:, r0 : r0 + 32, :],
                op0=ALU.mult,
                op1=ALU.mult,
            )
            nc.sync.dma_start(out_d.ap()[:, r0 : r0 + 32, :], out_t[:, r0 : r0 + 32, :])

    nc.compile()
    nc.m = get_hw_module(nc.m)
    return nc


_PROGRAM = None


def _get_program():
    global _PROGRAM
    if _PROGRAM is None:
        _PROGRAM = build_program()
    return _PROGRAM


def _prep_inputs(feat, src, Wq, bq, Wv, bv):
    bf = ml_dtypes.bfloat16
    # src parity planes: (B, ct, p, q, 128, H, W)
    spl = np.ascontiguousarray(
        src.reshape(B, 2, 128, H, 2, W, 2).transpose(0, 1, 4, 6, 2, 3, 5)
    ).astype(bf)
    featb = feat.reshape(B, 2, 128, H, W).astype(bf)
    identb = np.eye(128, dtype=np.float32).astype(bf)
    # lhsT layout [ct_in, cin_local, cout]: Wq.T[cin, cout] split over cin
    wq3 = np.ascontiguousarray(Wq.T).reshape(2, 128, C)
    wv3 = np.ascontiguousarray(Wv.T).reshape(2, 128, C)
    in_maps = []
    for core in range(N_CORES):
        b, h = divmod(core, 2)
        oc = slice(h * 128, h * 128 + 128)
        order = [h, 1 - h]  # slot 0 = own input-channel half
        in_maps.append(
            dict(
                featb=np.ascontiguousarray(featb[b][order]),
                featf=np.ascontiguousarray(feat[b, oc]).reshape(128, H, W),
                splanes=np.ascontiguousarray(spl[b][order]),
                wq=np.ascontiguousarray(wq3[order][:, :, oc]).astype(bf),
                wv=np.ascontiguousarray(wv3[order][:, :, oc]).astype(bf),
                bq=bq[oc].reshape(128, 1).astype(np.float32),
                bv=bv[oc].reshape(128, 1).astype(np.float32),
                identb=identb,
            )
        )
    return in_maps


def kernel(feat, src, Wq, bq, Wv, bv, _trace=False):
    feat = np.asarray(feat, np.float32)
    src = np.asarray(src, np.float32)
    Wq = np.asarray(Wq, np.float32)
    bq = np.asarray(bq, np.float32)
    Wv = np.asarray(Wv, np.float32)
    bv = np.asarray(bv, np.float32)

    in_maps = _prep_inputs(feat, src, Wq, bq, Wv, bv)
    nc = _get_program()
    res = bass_utils.run_bass_kernel_spmd(
        nc, in_maps, core_ids=list(range(N_CORES)), trace=_trace
    )
    out = np.empty((B, C, H, W), np.float32)
    for core in range(N_CORES):
        b, h = divmod(core, 2)
        out[b, h * 128 : h * 128 + 128] = res.results[core]["out"]
    if _trace:
        kernel.last_results = res
    return out


kernel.last_results = None


# revision 7
# speedup vs baseline: 1.0927x; 1.0927x over previous
"""Trainium2 Bass kernel for nn_CRAP_16544214024675 (sparse_attention).

Reference computation (per batch b, channel c):
  q = Wq@feat + bq                        (1x1 conv over channels)
  k = unfold3x3_s2(src)                   (strided window gather, pad 1)
  v = unfold3x3_s2(Wv@src + bv)
  A = softmax_t( sum_px q*k_t / 64 )      (9 window positions)
  out = fold3x3_s1( A_t * v_t ) * feat

Sharding: 8 cores = 4 batches x 2 output-channel halves. The per-core
program is identical: inputs are packed so channel-slot 0 is always the
core's OWN half (host reorders the contraction; channel sums commute).

Layout trick: unfold/fold never materialize. With parity planes
P[p,q][c,h,w] = x[c, 2h+p, 2w+q]:
  k_(i,j)[h,w] = src[2h+i-1, 2w+j-1] -> srcplane[(i+1)%2][(j+1)%2]
    shifted by (i==0 ? -1 : 0, j==0 ? -1 : 0)
  fold sample for t=(i,j) at (y,x) = vsrc[2y+1-i, 2x+1-j]
    -> vplane[(i+1)%2][(j+1)%2][y + (i==2 ? -1 : 0), x + (j==2 ? -1 : 0)]
Out-of-range samples are the zero-pad terms; every op restricts to its
valid window so they drop out exactly (incl. the fold's dropped
(y=63,i=0)/(x=63,j=0) terms; PSUM has_written handles partial windows
because the full-window t=(1,1) runs first with start=True).

Engines:
  PE : warm-up burst, q-conv, v-conv, fold (diag(exp_t) stationary,
       PSUM-accumulated over the 9 t's)
  DVE: logits = scalar_tensor_tensor (q*k_t with fused accum_out sum),
       diag builds, 1/Z, final (fold*1/Z)*feat fused multiply
  ACT: PSUM->SBUF copies with fused bias (Identity), exp(logit/64),
       q column-shift copy
DMA is issued from sync+gpsimd in consumer order (feat, then src planes
in first-use order, featf last).
"""
import sys
from contextlib import ExitStack

import numpy as np

for _p in ("/opt/trn_rl_repo", "/root/.axon_site/_ro/trn_rl_repo"):
    if _p not in sys.path:
        sys.path.append(_p)

import ml_dtypes

import concourse.tile as tile
from concourse import bacc, mybir
from concourse import bass_utils
from concourse.bass_interp import get_hw_module

F32 = mybir.dt.float32
BF16 = mybir.dt.bfloat16
AF = mybir.ActivationFunctionType
ALU = mybir.AluOpType

B, C, H, W = 4, 256, 64, 64
N_CORES = 8

# (1,1) first (full-window fold matmul initializes PSUM with start=True);
# the three j==0 entries (which need the shifted q copy) go last so the
# ACT-produced shift is ready off the critical path.
T_ORDER = [(1, 1), (0, 1), (0, 2), (1, 2), (2, 1), (2, 2), (0, 0), (1, 0), (2, 0)]
# srcplane/vplane first-use order for t in T_ORDER: (p,q)=((i+1)%2,(j+1)%2)
PLANE_ORDER = [(0, 0), (1, 0), (1, 1), (0, 1)]


def build_program():
    nc = bacc.Bacc("TRN2", target_bir_lowering=False, debug=False)

    featb_d = nc.dram_tensor("featb", (2, 128, H, W), BF16, kind="ExternalInput")
    featf_d = nc.dram_tensor("featf", (128, H, W), F32, kind="ExternalInput")
    spl_d = nc.dram_tensor("splanes", (2, 2, 2, 128, H, W), BF16, kind="ExternalInput")
    wq_d = nc.dram_tensor("wq", (2, 128, 128), BF16, kind="ExternalInput")
    wv_d = nc.dram_tensor("wv", (2, 128, 128), BF16, kind="ExternalInput")
    bq_d = nc.dram_tensor("bq", (128, 1), F32, kind="ExternalInput")
    bv_d = nc.dram_tensor("bv", (128, 1), F32, kind="ExternalInput")
    id_d = nc.dram_tensor("identb", (128, 128), BF16, kind="ExternalInput")
    out_d = nc.dram_tensor("out", (128, H, W), F32, kind="ExternalOutput")

    with tile.TileContext(nc) as tc, ExitStack() as ctx:
        pool = ctx.enter_context(tc.tile_pool(name="main", bufs=1))
        scpool = ctx.enter_context(tc.tile_pool(name="scratch", bufs=1))
        dgpool = ctx.enter_context(tc.tile_pool(name="diags", bufs=9))

        # --- weights / constants on scalar+vector HWDGE queues (gpsimd's
        # software queue stalls ~2.6us DRAIN per issue - avoid it) ---
        wq_t = pool.tile([128, 2, 128], BF16, tag="wq")
        wv_t = pool.tile([128, 2, 128], BF16, tag="wv")
        nc.scalar.dma_start(wq_t[:], wq_d.ap().rearrange("a p b -> p a b"))
        nc.scalar.dma_start(wv_t[:], wv_d.ap().rearrange("a p b -> p a b"))
        bq_t = pool.tile([128, 1], F32, tag="bq")
        nc.scalar.dma_start(bq_t[:], bq_d.ap())
        bv_t = pool.tile([128, 1], F32, tag="bv")
        nc.scalar.dma_start(bv_t[:], bv_d.ap())
        id_t = pool.tile([128, 128], BF16, tag="identb")
        nc.scalar.dma_start(id_t[:], id_d.ap())

        # --- feat (bf16, slot 0 = own half) ---
        featb_t = []
        for k in range(2):
            t_ = pool.tile([128, H, W], BF16, tag=f"featb{k}")
            (nc.sync if k == 0 else nc.scalar).dma_start(t_[:], featb_d.ap()[k])
            featb_t.append(t_)

        # --- src parity planes, first-use order, split across HWDGE queues ---
        splane = [[[None] * 2 for _ in range(2)] for _ in range(2)]
        eng_rr = [nc.sync, nc.scalar]
        n_dma = 0
        for (p, q) in PLANE_ORDER:
            for ct in range(2):
                t_ = pool.tile(
                    [128, H, W], BF16, tag=f"spl{ct}{p}{q}", name=f"spl{ct}{p}{q}"
                )
                eng_rr[n_dma % 2].dma_start(t_[:], spl_d.ap()[ct, p, q])
                splane[ct][p][q] = t_
                n_dma += 1

        with tc.tile_pool(name="psq", bufs=2, space="PSUM") as psq:
            # --- PE warm-up: ~5us of dummy matmuls so the HAM clock-gate
            # opens before the real convs (weights land in ~1us) ---
            wps = psq.tile([128, 32, W], F32, tag="ps", name="warmps")
            for w_i in range(24):
                nc.tensor.matmul(
                    wps[:, 0:4, :],
                    wq_t[:, 0, :],
                    wq_t[:].rearrange("p a b -> p (a b)"),
                    start=True,
                    stop=True,
                    skip_group_check=True,
                )

            # --- q-conv: q = Wq@feat + bq -> bf16 (bias fused on ACT copy) ---
            q_t = pool.tile([128, H, W], BF16, tag="q")
            qs_t = pool.tile([128, H, W], BF16, tag="qs")
            for half in range(2):
                ps = psq.tile([128, 32, W], F32, tag="ps")
                r0 = 32 * half
                for s in range(4):
                    for k in range(2):
                        nc.tensor.matmul(
                            ps[:, 8 * s : 8 * s + 8, :],
                            wq_t[:, k, :],
                            featb_t[k][:, r0 + 8 * s : r0 + 8 * s + 8, :],
                            start=(k == 0),
                            stop=(k == 1),
                        )
                nc.scalar.activation(
                    q_t[:, r0 : r0 + 32, :], ps[:], AF.Identity, bias=bq_t[:]
                )
            # shifted q copy (ACT; used only by the three late j==0 t's)
            nc.scalar.activation(qs_t[:, :, 0:63], q_t[:, :, 1:64], AF.Copy)

            # --- v-conv: vplane[p][q] = Wv@srcplane[:, p, q] + bv ---
            vplane = [[None] * 2 for _ in range(2)]
            for (p, q) in PLANE_ORDER:
                vplane[p][q] = pool.tile(
                    [128, H, W], BF16, tag=f"vpl{p}{q}", name=f"vpl{p}{q}"
                )
                for half in range(2):
                    ps = psq.tile([128, 32, W], F32, tag="ps")
                    r0 = 32 * half
                    for s in range(4):
                        for k in range(2):
                            nc.tensor.matmul(
                                ps[:, 8 * s : 8 * s + 8, :],
                                wv_t[:, k, :],
                                splane[k][p][q][:, r0 + 8 * s : r0 + 8 * s + 8, :],
                                start=(k == 0),
                                stop=(k == 1),
                            )
                    nc.scalar.activation(
                        vplane[p][q][:, r0 : r0 + 32, :],
                        ps[:],
                        AF.Identity,
                        bias=bv_t[:],
                    )

        # --- per-t pipeline: fused logit (DVE) -> exp (ACT) -> diag (DVE)
        #     -> fold matmuls (PE, PSUM-accumulated) ---
        psf = ctx.enter_context(tc.tile_pool(name="psf", bufs=1, space="PSUM"))
        lg_t = pool.tile([128, 9], F32, tag="lg")
        exp_t = pool.tile([128, 9], F32, tag="exp")
        fold_ps = psf.tile([128, H, W], F32, tag="fold")
        sc = scpool.tile([128, H, W], BF16, tag="prod")

        for idx, (i, j) in enumerate(T_ORDER):
            if i == 0:
                qr0, rows = 1, 63
            else:
                qr0, rows = 0, 64
            if j == 0:
                qq, cols = qs_t, 63
            else:
                qq, cols = q_t, 64
            pl = splane[0][(i + 1) % 2][(j + 1) % 2]
            nc.vector.scalar_tensor_tensor(
                out=sc[:, 0:rows, 0:cols],
                in0=qq[:, qr0 : qr0 + rows, 0:cols],
                scalar=1.0,
                in1=pl[:, 0:rows, 0:cols],
                op0=ALU.mult,
                op1=ALU.mult,
                accum_out=lg_t[:, idx : idx + 1],
            )
            nc.scalar.activation(
                exp_t[:, idx : idx + 1],
                lg_t[:, idx : idx + 1],
                AF.Exp,
                scale=1.0 / 64.0,
            )
            dg = dgpool.tile([128, 128], BF16, tag="diag", name=f"diag{idx}")
            nc.scalar.activation(
                dg[:], id_t[:], AF.Identity, scale=exp_t[:, idx : idx + 1]
            )

            # fold windows: psum[y,x] += exp_t * vplane[pi][pj][y+dy, x+dx]
            if i == 0:
                yo0, yo1, dy = 0, 63, 0
            elif i == 1:
                yo0, yo1, dy = 0, 64, 0
            else:
                yo0, yo1, dy = 1, 64, -1
            if j == 0:
                xo0, xo1, dx = 0, 63, 0
            elif j == 1:
                xo0, xo1, dx = 0, 64, 0
            else:
                xo0, xo1, dx = 1, 64, -1
            vp = vplane[(i + 1) % 2][(j + 1) % 2]
            yb = yo0
            while yb < yo1:
                ye = min((yb // 8 + 1) * 8, yo1)
                nc.tensor.matmul(
                    fold_ps[:, yb:ye, xo0:xo1],
                    dg[:],
                    vp[:, yb + dy : ye + dy, xo0 + dx : xo1 + dx],
                    start=(idx == 0),
                    stop=(idx == 8),
                    skip_group_check=True,
                )
                yb = ye

        # --- 1/Z off the critical path ---
        z_t = pool.tile([128, 1], F32, tag="z")
        rz_t = pool.tile([128, 1], F32, tag="rz")
        nc.vector.tensor_reduce(z_t[:], exp_t[:], axis=mybir.AxisListType.X, op=ALU.add)
        nc.vector.reciprocal(rz_t[:], z_t[:])

        # --- final: out = (fold * 1/Z) * feat, two halves pipelined ---
        fs_t = pool.tile([128, H, W], F32, tag="fs")
        nc.scalar.dma_start(fs_t[:], featf_d.ap())
        out_t = pool.tile([128, H, W], F32, tag="out")
        for half in range(2):
            r0 = 32 * half
            nc.vector.scalar_tensor_tensor(
                out=out_t[:, r0 : r0 + 32, :],
                in0=fold_ps[:, r0 : r0 + 32, :],
                scalar=rz_t[:],
                in1=fs_t[:, r0 : r0 + 32, :],
                op0=ALU.mult,
                op1=ALU.mult,
            )
            nc.sync.dma_start(out_d.ap()[:, r0 : r0 + 32, :], out_t[:, r0 : r0 + 32, :])

    nc.compile()
    nc.m = get_hw_module(nc.m)
    return nc


_PROGRAM = None


def _get_program():
    global _PROGRAM
    if _PROGRAM is None:
        _PROGRAM = build_program()
    return _PROGRAM


def _prep_inputs(feat, src, Wq, bq, Wv, bv):
    bf = ml_dtypes.bfloat16
    # src parity planes: (B, ct, p, q, 128, H, W)
    spl = np.ascontiguousarray(
        src.reshape(B, 2, 128, H, 2, W, 2).transpose(0, 1, 4, 6, 2, 3, 5)
    ).astype(bf)
    featb = feat.reshape(B, 2, 128, H, W).astype(bf)
    identb = np.eye(128, dtype=np.float32).astype(bf)
    # lhsT layout [ct_in, cin_local, cout]: Wq.T[cin, cout] split over cin
    wq3 = np.ascontiguousarray(Wq.T).reshape(2, 128, C)
    wv3 = np.ascontiguousarray(Wv.T).reshape(2, 128, C)
    in_maps = []
    for core in range(N_CORES):
        b, h = divmod(core, 2)
        oc = slice(h * 128, h * 128 + 128)
        order = [h, 1 - h]  # slot 0 = own input-channel half
        in_maps.append(
            dict(
                featb=np.ascontiguousarray(featb[b][order]),
                featf=np.ascontiguousarray(feat[b, oc]).reshape(128, H, W),
                splanes=np.ascontiguousarray(spl[b][order]),
                wq=np.ascontiguousarray(wq3[order][:, :, oc]).astype(bf),
                wv=np.ascontiguousarray(wv3[order][:, :, oc]).astype(bf),
                bq=bq[oc].reshape(128, 1).astype(np.float32),
                bv=bv[oc].reshape(128, 1).astype(np.float32),
                identb=identb,
            )
        )
    return in_maps


def kernel(feat, src, Wq, bq, Wv, bv, _trace=False):
    feat = np.asarray(feat, np.float32)
    src = np.asarray(src, np.float32)
    Wq = np.asarray(Wq, np.float32)
    bq = np.asarray(bq, np.float32)
    Wv = np.asarray(Wv, np.float32)
    bv = np.asarray(bv, np.float32)

    in_maps = _prep_inputs(feat, src, Wq, bq, Wv, bv)
    nc = _get_program()
    res = bass_utils.run_bass_kernel_spmd(
        nc, in_maps, core_ids=list(range(N_CORES)), trace=_trace
    )
    out = np.empty((B, C, H, W), np.float32)
    for core in range(N_CORES):
        b, h = divmod(core, 2)
        out[b, h * 128 : h * 128 + 128] = res.results[core]["out"]
    if _trace:
        kernel.last_results = res
    return out


kernel.last_results = None
